# revision 20
# baseline (speedup 1.0000x reference)
"""Trainium2 Bass kernel for an encoder-decoder LSTM (seq2seq), fp8 edition.

Model (see problem reference):
  B=1024, S=96, I=128; HE=HD=1024, O=128, HORIZON=24
  encoder: 96-step LSTMCell(I=128 -> H=1024)
  bridge:  h_dec = h_enc @ W_fc.T + b_fc ; c_dec = h_enc @ W_fcc.T + b_fcc
  decoder: 24 autoregressive LSTMCell(O=128 -> H=1024) steps + out = h @ W_out.T + b_out

Sharding: data-parallel over batch, B=1024 -> 128 per core on 8 cores.
Weights replicated; recurrence runs locally per core; no collectives.

Per-core layout ("batch stationary, weights moving"), fp8 DoubleRow:
  - all recurrent matmuls run in fp8e4 (TRN e4m3, max +-240) with
    perf_mode=DoubleRow: each matmul contracts TWO 128-row K-chunks at once
    (stationary [K=128, 2, M], moving [K=128, 2, N=512]), 2x the fp16 rate.
  - scales: weights * 4096, activations (h, x) * 32; PSUM is scaled by
    2^17 and the ACT engine un-scales during the fused PSUM->activation
    evacuation (activation(scale=2^-17) reads PSUM directly).
  - biases are folded into the matmul: the (x | bias) DoubleRow pair uses a
    stationary bias chunk whose rows 0,1 hold the constant A=64 and whose
    moving rows hold an exact-ish hi/lo fp8 split of b * 2^17 / 64.
    This removes the DVE evacuation pass entirely.
  - h is produced in fp16 slices, transposed (slices 0-2 PE, 3-7 DMA; DMA
    transpose needs 2-byte dtypes) and cast to fp8*32 for the next step's
    stationary operand (the PE-transpose evac cast is fused into the ACT
    copy; DMA-transposed slices get two grouped DVE casts).
  - the decoder output projection y = h @ W_out.T stays fp16 (it feeds the
    tiny-magnitude output directly; fp8 there doubles the final error), so
    the decoder also keeps an fp16 hT copy.
  - decoder weights are prefetched during the encoder (fp8 halves their
    footprint; both weight sets fit in SBUF), removing the bridge stall.

Measured numerics (numpy sim of this exact scheme): rel_err ~6e-3 vs the
fp32 reference (tolerance 2e-2).
"""

import numpy as np

# ---- model dims (hardcoded; kernel.py must be self-contained) ----
B, S, I = 1024, 96, 128
H = 1024          # HE == HD
O = 128
HORIZON = 24
NCORES = 8
BC = B // NCORES  # 128 batch rows per core
P = 128           # partitions
KH = H // P       # 8 hidden K-chunks
NQ = KH // 2 + 1  # 4 DoubleRow h-pairs + 1 (x | bias) pair
G = 4 * H         # 4096 gate columns, torch order [i | f | g | o]

# ---- fp8 scale scheme ----
SW = 4096.0       # weight scale (W_hh, W_fc, W_fcc, W_ih_e)
SH = 32.0         # h / x activation scale
SS = SW * SH      # PSUM scale = 2^17
SC = 1.0 / SS
SINP = 512.0      # decoder y-feedback scale
SWID = SS / SINP  # decoder W_ih_d scale = 256
ABIAS = 64.0      # bias stationary constant (rows 0,1)


def _build_bass(s_steps=S, horizon=HORIZON):
    import concourse.bass as bass
    import concourse.tile as tile
    from concourse import bacc, mybir

    f8 = mybir.dt.float8e4
    f16 = mybir.dt.float16
    f32 = mybir.dt.float32
    MULT = mybir.AluOpType.mult
    ADD = mybir.AluOpType.add
    SIG = mybir.ActivationFunctionType.Sigmoid
    TANH = mybir.ActivationFunctionType.Tanh
    IDENT = mybir.ActivationFunctionType.Identity
    DR = mybir.MatmulPerfMode.DoubleRow

    import os

    nc = bacc.Bacc()
    _trace_sim = os.environ.get("BASS_TRACE_SIM", "0") == "1"

    xT_d = nc.dram_tensor("xT", [s_steps, I, BC], f8, kind="ExternalInput")
    ew_d = nc.dram_tensor("ew", [P, NQ, 2, G], f8, kind="ExternalInput")
    dw_d = nc.dram_tensor("dw", [P, NQ, 2, G], f8, kind="ExternalInput")
    bw_d = nc.dram_tensor("bw", [P, KH // 2, 2, 2 * H], f8, kind="ExternalInput")
    ow_d = nc.dram_tensor("ow", [P, KH, O], f16, kind="ExternalInput")
    bbr_d = nc.dram_tensor("bbr", [2 * H], f32, kind="ExternalInput")
    bo_d = nc.dram_tensor("bo", [O], f32, kind="ExternalInput")
    y_d = nc.dram_tensor("y", [horizon, O, BC], f32, kind="ExternalOutput")

    def bcast_rows(ap):
        # DRAM [N] -> read the same row on all 128 partitions
        return bass.AP(tensor=ap.tensor, offset=ap.offset, ap=[[0, P], *ap.ap])

    with tile.TileContext(nc, trace_sim=_trace_sim) as tc:
        with (
            tc.tile_pool(name="consts", bufs=1) as consts,
            tc.tile_pool(name="wpool", bufs=1) as wpool,
            tc.tile_pool(name="state", bufs=1) as state,
            tc.tile_pool(name="hpool", bufs=2) as hpool,
            tc.tile_pool(name="ypool", bufs=2) as ypool,
            tc.tile_pool(name="gpsum", bufs=3, space="PSUM") as gpsum,
            tc.tile_pool(name="trpool", bufs=2, space="PSUM") as trpool,
        ):
            # ---- encoder weights first: the prologue needs the q=4 pair;
            # x[0] jumps the queue so the first matmuls can start early ----
            xb = [state.tile([P, 2, BC], f8, name=f"xb{i}") for i in range(2)]
            nc.sync.dma_start(out=xb[0][:, 0, :], in_=xT_d[0])
            ew_sb = wpool.tile([P, NQ, 2, G], f8, tag="we", name="we")
            for q in (NQ - 1, *range(NQ - 1)):
                nc.sync.dma_start(out=ew_sb[:, q], in_=ew_d[:, q])
            # decoder weights: chunks issued inside encoder steps (prefetch)
            dw_sb = wpool.tile([P, NQ, 2, G], f8, tag="wd", name="wd")

            # ---- constants (scalar/gpsimd queues, off the critical path) ----
            bbr_sb = consts.tile([P, 2 * H], f32)
            nc.gpsimd.dma_start(out=bbr_sb, in_=bcast_rows(bbr_d[:]))
            bo_sb = consts.tile([P, 1], f32)
            nc.scalar.dma_start(out=bo_sb, in_=bo_d[:][:, None])
            bw_sb = consts.tile([P, KH // 2, 2, 2 * H], f8)
            nc.scalar.dma_start(out=bw_sb, in_=bw_d[:])
            ow_sb = consts.tile([P, KH, O], f16)
            nc.scalar.dma_start(out=ow_sb, in_=ow_d[:])
            ident = consts.tile([P, P], f16)
            from concourse.masks import make_identity
            make_identity(nc, ident)

            # x / feedback stationary tiles: [P, 2, BC] fp8 where ko=0 is the
            # per-step input chunk and ko=1 is the static bias chunk
            # (rows 0,1 = ABIAS, rest 0)
            dxb = state.tile([P, 2, BC], f8, name="dxb")
            for t_ in (*xb, dxb):
                nc.vector.memset(t_[:, 1, :], 0.0)
                nc.vector.memset(t_[0:2, 1, :], ABIAS)
            nc.vector.memset(dxb[:, 0, :], 0.0)  # decoder t=0 input is zero

            # ---- persistent state ----
            c_sb = state.tile([P, H], f32)       # cell state, [B, H]
            pre = state.tile([P, G], f32)        # gate post-activations
            cf = state.tile([P, H], f32)
            ig = state.tile([P, H], f32)
            thc = state.tile([P, H], f32)        # tanh(c)

            def alloc_pair():
                return gpsum.tile([P, H], f32, tag="g", name="gps")

            def emit_xb(w, pst, pair, xbt, hh, start, stop):
                # (x | bias) DoubleRow pair for one gate, one 512-tile
                col = pair * H + hh * 512
                nc.tensor.matmul(
                    pst[:, hh * 512 : hh * 512 + 512],
                    lhsT=xbt[:], rhs=w[:, NQ - 1, :, col : col + 512],
                    start=start, stop=stop, perf_mode=DR,
                )

            def emit_h_j(w, pst, pair, hT8, j, hh, start, stop):
                # one DoubleRow matmul (h chunks 2j, 2j+1), one 512-tile
                col = pair * H + hh * 512
                nc.tensor.matmul(
                    pst[:, hh * 512 : hh * 512 + 512],
                    lhsT=hT8[:, 2 * j : 2 * j + 2, :],
                    rhs=w[:, j, :, col : col + 512],
                    start=start, stop=stop, perf_mode=DR,
                )

            def emit_if_pairs(w, ps, hT8, start, stop, po_hook=None,
                              tr_hook=None):
                """Gate pairs i and f, j-interleaved so hT8 chunk pairs are
                consumed at the rate the previous step's tail produces them.
                tr_hook (after j0) emits the previous step's late PE
                transposes of h slices 4,5 -- their casts land just before
                j2 consumes chunk pair (4,5)."""
                for j in range(KH // 2):
                    for pair in (0, 1):
                        for hh in range(2):
                            emit_h_j(w, ps[pair], pair, hT8, j, hh,
                                     start=(start and j == 0),
                                     stop=(stop and j == KH // 2 - 1))
                    if po_hook is not None:
                        po_hook(j)
                    if tr_hook is not None:
                        tr_hook(j)

            def emit_go_phase(w, pst, pair, hT8, xbt, hh, xb_start):
                """One 512-tile of gate g or o: xb pair + 4 h pairs, stopping
                this PSUM region as early as possible (the cell tail reads
                each region right after its stop)."""
                emit_xb(w, pst, pair, xbt, hh, start=xb_start, stop=hT8 is None)
                if hT8 is not None:
                    for j in range(KH // 2):
                        emit_h_j(w, pst, pair, hT8, j, hh,
                                 start=False, stop=(j == KH // 2 - 1))

            # slice bounds for the cell tail: slices 0-3 live in the hh0
            # PSUM regions (stop early), slices 4-7 in hh1 (stop last)
            S01 = slice(0, 2 * P)
            S23 = slice(2 * P, 4 * P)
            SLH = slice(4 * P, H)

            def emit_cell(ps, first_cell, want_h16):
                """PSUM gate pairs -> new c (in place), h slices (fp16) and
                the transposed hT8 (+hT16 for the decoder) for the next step.

                ACT evacuates PSUM directly with the fused 2^-17 un-scale
                (bias was matmul-accumulated). ACT queue (11 ops/step):
                i, f full-width; g/tanh(c)/o in s01 / s23 / sh slices, in
                that order -- the s01 chain completes while the o-hh1
                matmuls still stream, so the PE transposes of slices 0-3
                (queued after the next step's xb matmuls) never stall.
                Slices 4-6 are PE-transposed from hooks inside the next
                step's j-loop (after j1 / j2), slice 7 rides the sync DMA
                queue; every hT8 cast lands just before its chunk pair is
                consumed.
                """
                def act_ps(lo, width, psrc, func):
                    nc.scalar.activation(
                        out=pre[:, lo : lo + width], in_=psrc, func=func,
                        scale=SC,
                    )

                def ig_c(sl):
                    nc.vector.tensor_tensor(
                        out=ig[:, sl], in0=pre[:, sl],
                        in1=pre[:, 2 * H + sl.start : 2 * H + sl.stop], op=MULT,
                    )
                    if first_cell:
                        nc.vector.tensor_copy(out=c_sb[:, sl], in_=ig[:, sl])
                    else:
                        nc.vector.tensor_tensor(
                            out=c_sb[:, sl], in0=cf[:, sl], in1=ig[:, sl], op=ADD
                        )

                def cf_upd(sl):
                    if not first_cell:
                        nc.gpsimd.tensor_tensor(
                            out=cf[:, sl], in0=pre[:, H + sl.start : H + sl.stop],
                            in1=c_sb[:, sl], op=MULT,
                        )

                ob = 3 * H
                h_sb = hpool.tile([P, H], f16, tag="h", name="hsb")
                hT8n = hpool.tile([P, KH, BC], f8, tag="hT8", name="hT8")
                hT16n = hpool.tile([P, KH, BC], f16, tag="hT16", name="hT16")

                def h_slice(s, eng):
                    sl = slice(s * P, (s + 1) * P)
                    eng.tensor_tensor(
                        out=h_sb[:, sl], in0=pre[:, ob + s * P : ob + (s + 1) * P],
                        in1=thc[:, sl], op=MULT,
                    )

                def tr_evac(s):
                    pst = trpool.tile([P, P], f16, tag="t", name="trp")
                    nc.tensor.transpose(
                        pst[:], h_sb[:, s * P : (s + 1) * P], ident
                    )
                    nc.vector.tensor_scalar_mul(hT8n[:, s, :], pst[:], SH)
                    if want_h16:
                        nc.vector.tensor_copy(out=hT16n[:, s, :], in_=pst[:])

                # ---- i, f full-width (their PSUM stops first) ----
                act_ps(0, H, ps[0][:], SIG)
                if not first_cell:
                    act_ps(H, H, ps[1][:], SIG)
                # ---- s01 chain (hh0 regions of g, o) ----
                act_ps(2 * H, 2 * P, ps[2][:, S01], TANH)
                cf_upd(S01)
                ig_c(S01)
                nc.scalar.activation(out=thc[:, S01], in_=c_sb[:, S01], func=TANH)
                act_ps(ob, 2 * P, ps[3][:, S01], SIG)
                h_slice(0, nc.vector)
                h_slice(1, nc.vector)
                # ---- s23 chain ----
                act_ps(2 * H + 2 * P, 2 * P, ps[2][:, S23], TANH)
                cf_upd(S23)
                ig_c(S23)
                nc.scalar.activation(out=thc[:, S23], in_=c_sb[:, S23], func=TANH)
                act_ps(ob + 2 * P, 2 * P, ps[3][:, S23], SIG)
                h_slice(2, nc.vector)
                h_slice(3, nc.vector)
                # ---- sh chain (hh1 regions, stop last) ----
                act_ps(2 * H + 4 * P, H - 4 * P, ps[2][:, SLH], TANH)
                cf_upd(SLH)
                ig_c(SLH)
                nc.scalar.activation(out=thc[:, SLH], in_=c_sb[:, SLH], func=TANH)
                act_ps(ob + 4 * P, H - 4 * P, ps[3][:, SLH], SIG)
                h_slice(4, nc.vector)
                h_slice(5, nc.vector)
                h_slice(6, nc.gpsimd)
                h_slice(7, nc.gpsimd)
                # slice 7: the only DMA transpose (sync queue; a single
                # transpose per step also avoids the concurrent-xbar race)
                nc.sync.dma_start(
                    out=hT16n[:, 7, :], in_=h_sb[:, 7 * P : H], transpose=True,
                )
                nc.vector.tensor_scalar_mul(hT8n[:, 7, :], hT16n[:, 7, :], SH)

                # slices 0-3: PE transposes, queued after the next step's xb
                # matmuls (h0-h3 are ready by the time the PE reaches them)
                for s in range(4):
                    tr_evac(s)

                def late_tr(j):
                    # called from the NEXT step's j-loop: j==1 -> slices 4,5
                    # (casts land before j2 consumes chunk pair (4,5));
                    # j==2 -> slice 6
                    if j == 1:
                        tr_evac(4)
                        tr_evac(5)
                    elif j == 2:
                        tr_evac(6)

                return hT8n, hT16n, late_tr

            # ================= encoder =================
            ps_cur = {p: alloc_pair() for p in (0, 1)}
            for p in (0, 1):
                for hh in range(2):
                    emit_xb(ew_sb, ps_cur[p], p, xb[0], hh, start=True, stop=True)
            hT8 = None
            late_tr = None
            dw_loaded = 0
            for t in range(s_steps):
                first = t == 0
                # decoder weight prefetch, one chunk every other step
                if t >= 2 and t % 2 == 0 and dw_loaded < NQ:
                    nc.sync.dma_start(out=dw_sb[:, dw_loaded], in_=dw_d[:, dw_loaded])
                    dw_loaded += 1
                if not first:
                    emit_if_pairs(ew_sb, ps_cur, hT8, start=False, stop=True,
                                  tr_hook=late_tr)
                ps_cur[2] = alloc_pair()
                ps_cur[3] = alloc_pair()
                # g/o phases with early per-tile PSUM stops:
                # g-hh0, o-hh0, g-hh1, o-hh1 (o last: shortest tail)
                for pair, hh in ((2, 0), (3, 0), (2, 1), (3, 1)):
                    emit_go_phase(ew_sb, ps_cur[pair], pair, hT8, xb[t % 2],
                                  hh, xb_start=True)
                if t + 1 < s_steps:
                    nxb = xb[(t + 1) % 2]
                    nc.sync.dma_start(out=nxb[:, 0, :], in_=xT_d[t + 1])
                    ps_next = {p: alloc_pair() for p in (0, 1)}
                    for p in (0, 1):
                        for hh in range(2):
                            emit_xb(ew_sb, ps_next[p], p, nxb, hh,
                                    start=True, stop=False)
                else:
                    ps_next = None
                hT8, _, late_tr = emit_cell(ps_cur, first_cell=first,
                                            want_h16=False)
                ps_cur = ps_next

            # ================= bridge =================
            # any decoder-weight chunks the (short) encoder didn't cover
            while dw_loaded < NQ:
                nc.sync.dma_start(out=dw_sb[:, dw_loaded], in_=dw_d[:, dw_loaded])
                dw_loaded += 1
            ps_h = alloc_pair()
            ps_c = alloc_pair()
            for j in range(KH // 2):
                for hh in range(2):
                    nc.tensor.matmul(
                        ps_h[:, hh * 512 : hh * 512 + 512],
                        lhsT=hT8[:, 2 * j : 2 * j + 2, :],
                        rhs=bw_sb[:, j, :, hh * 512 : hh * 512 + 512],
                        start=(j == 0), stop=(j == KH // 2 - 1), perf_mode=DR,
                    )
                    nc.tensor.matmul(
                        ps_c[:, hh * 512 : hh * 512 + 512],
                        lhsT=hT8[:, 2 * j : 2 * j + 2, :],
                        rhs=bw_sb[:, j, :, H + hh * 512 : H + hh * 512 + 512],
                        start=(j == 0), stop=(j == KH // 2 - 1), perf_mode=DR,
                    )
                if late_tr is not None:
                    late_tr(j)
            nc.vector.scalar_tensor_tensor(
                out=c_sb[:], in0=ps_c[:], scalar=SC, in1=bbr_sb[:, H : 2 * H],
                op0=MULT, op1=ADD,
            )
            h_sb = hpool.tile([P, H], f16, tag="h", name="hsb")
            nc.vector.scalar_tensor_tensor(
                out=h_sb[:], in0=ps_h[:], scalar=SC, in1=bbr_sb[:, 0:H],
                op0=MULT, op1=ADD,
            )
            # bridge h transposes on the (idle) PE; only hdT8 is needed
            # downstream (the projection never reads the bridge h)
            hT8 = hpool.tile([P, KH, BC], f8, tag="hT8", name="hT8")
            for s in range(KH):
                pst = trpool.tile([P, P], f16, tag="t", name="trp")
                nc.tensor.transpose(pst[:], h_sb[:, s * P : (s + 1) * P], ident)
                nc.vector.tensor_scalar_mul(hT8[:, s, :], pst[:], SH)
            hT16 = None

            # ================= decoder =================
            # reference order: cell first (inp from the previous step, zeros
            # at t=0), then project the NEW h: y[t] = h_{t+1} @ W_out.T + b_out
            # The fp16 projection matmuls interleave into the NEXT step's i/f
            # j-loop (they consume the previous step's hT16 chunks).
            pend_po = None       # (po_tile, hT16_tile) awaiting projection
            late_tr = None       # bridge hT8 was fully PE-transposed inline
            for t in range(horizon):
                first = t == 0
                ps = {p: alloc_pair() for p in (0, 1, 2, 3)}

                # previous step's late transposes first: the projection below
                # reads hT16 slices 4-6 that they produce
                if late_tr is not None:
                    late_tr(1)
                    late_tr(2)
                if pend_po is not None:
                    # project the previous h, then y -> feedback input
                    ppo, phT = pend_po
                    for k in range(KH):
                        nc.tensor.matmul(
                            ppo[:, 0:BC], lhsT=ow_sb[:, k, :], rhs=phT[:, k, :],
                            start=(k == 0), stop=(k == KH - 1),
                        )
                    y_sb = ypool.tile([P, BC], f32, tag="y", name="ysb")
                    nc.scalar.activation(
                        out=y_sb[:], in_=ppo[:, 0:BC], func=IDENT, bias=bo_sb[:]
                    )
                    nc.sync.dma_start(out=y_d[t - 1], in_=y_sb[:])
                    nc.gpsimd.tensor_scalar_mul(dxb[:, 0, :], y_sb[:], SINP)
                # encoder-style step: xb first (start), h j-loops, early stops
                for p in (0, 1):
                    for hh in range(2):
                        emit_xb(dw_sb, ps[p], p, dxb, hh, start=True, stop=False)
                emit_if_pairs(dw_sb, ps, hT8, start=False, stop=True)
                for pair, hh in ((2, 0), (3, 0), (2, 1), (3, 1)):
                    emit_go_phase(dw_sb, ps[pair], pair, hT8, dxb, hh,
                                  xb_start=True)
                hT8, hT16, late_tr = emit_cell(ps, first_cell=False,
                                               want_h16=True)
                pend_po = (trpool.tile([P, BC], f32, tag="t", name="po"), hT16)

            # final step's projection (flush the pending late transposes
            # first -- they produce hT16 slices 4-6)
            if late_tr is not None:
                late_tr(1)
                late_tr(2)
            ppo, phT = pend_po
            for k in range(KH):
                nc.tensor.matmul(
                    ppo[:, 0:BC], lhsT=ow_sb[:, k, :], rhs=phT[:, k, :],
                    start=(k == 0), stop=(k == KH - 1),
                )
            y_sb = ypool.tile([P, BC], f32, tag="y", name="ysb")
            nc.scalar.activation(
                out=y_sb[:], in_=ppo[:, 0:BC], func=IDENT, bias=bo_sb[:]
            )
            nc.sync.dma_start(out=y_d[horizon - 1], in_=y_sb[:])

    nc.compile()
    return nc


def _prepare_inputs(inputs, s_steps=S):
    import ml_dtypes

    E4 = ml_dtypes.float8_e4m3
    f16 = np.float16

    def q8(a, scale):
        return np.clip(np.asarray(a, np.float32) * scale, -240.0, 240.0).astype(E4)

    x = np.asarray(inputs["x"], np.float32)[:, :s_steps]
    W_ih_e = np.asarray(inputs["W_ih_e"], np.float32)
    W_hh_e = np.asarray(inputs["W_hh_e"], np.float32)
    W_ih_d = np.asarray(inputs["W_ih_d"], np.float32)
    W_hh_d = np.asarray(inputs["W_hh_d"], np.float32)
    W_fc = np.asarray(inputs["W_fc"], np.float32)
    W_fcc = np.asarray(inputs["W_fcc"], np.float32)
    W_out = np.asarray(inputs["W_out"], np.float32)
    be = np.asarray(inputs["b_ih_e"], np.float32) + np.asarray(inputs["b_hh_e"], np.float32)
    bd = np.asarray(inputs["b_ih_d"], np.float32) + np.asarray(inputs["b_hh_d"], np.float32)

    def pack_w8(W_hh, W_ih, bias, s_ih):
        # -> [P, NQ, 2, G] fp8: q<4 = (h chunk 2q, 2q+1) of W_hh.T * SW;
        # q=4 ko0 = W_ih.T * s_ih, ko1 = bias rows (0=hi, 1=lo of b*SS/ABIAS)
        w = np.zeros((P, NQ, 2, G), E4)
        whhT = W_hh.T  # [H, G]
        for q in range(KH // 2):
            for c in range(2):
                k = 2 * q + c
                w[:, q, c] = q8(whhT[k * P : (k + 1) * P], SW)
        w[:, NQ - 1, 0] = q8(W_ih.T, s_ih)
        t_ = np.clip(bias * SS / ABIAS, -1e9, 1e9)
        hi = np.clip(t_, -240, 240).astype(E4)
        lo = np.clip(t_ - hi.astype(np.float32), -240, 240).astype(E4)
        w[0, NQ - 1, 1] = hi
        w[1, NQ - 1, 1] = lo
        return w

    ew = pack_w8(W_hh_e, W_ih_e, be, SW)
    dw = pack_w8(W_hh_d, W_ih_d, bd, SWID)

    bw = np.zeros((P, KH // 2, 2, 2 * H), E4)
    brT = np.concatenate([W_fc.T, W_fcc.T], axis=1)  # [H, 2H]
    for q in range(KH // 2):
        for c in range(2):
            k = 2 * q + c
            bw[:, q, c] = q8(brT[k * P : (k + 1) * P], SW)

    owT = W_out.T  # [H, O]
    ow = np.ascontiguousarray(
        owT.reshape(KH, P, O).transpose(1, 0, 2).astype(f16)
    )
    bbr = np.concatenate([inputs["b_fc"], inputs["b_fcc"]]).astype(np.float32)
    bo = np.asarray(inputs["b_out"], np.float32)

    shared = dict(ew=ew, dw=dw, bw=bw, ow=ow, bbr=bbr, bo=bo)
    in_maps = []
    for c in range(NCORES):
        xc = x[c * BC : (c + 1) * BC]                    # [BC, S, I]
        xT = np.ascontiguousarray(
            np.clip(xc.transpose(1, 2, 0) * SH, -240, 240).astype(E4)
        )                                                # [S, I, BC] fp8 * 32
        in_maps.append(dict(shared, xT=xT))
    return in_maps


def run(inputs, trace=False, s_steps=S, horizon=HORIZON):
    """Build, run on 8 cores, gather. Returns (full_output, BassKernelResults)."""
    import sys

    try:
        import concourse  # noqa: F401
    except ImportError:
        sys.path.insert(0, "/opt/trn_rl_repo")
    from concourse.bass_utils import run_bass_kernel_spmd

    nc = _build_bass(s_steps=s_steps, horizon=horizon)
    in_maps = _prepare_inputs(inputs, s_steps=s_steps)
    res = run_bass_kernel_spmd(nc, in_maps, core_ids=list(range(NCORES)), trace=trace)
    out = np.empty((B, horizon, O), np.float32)
    for c in range(NCORES):
        yc = res.results[c]["y"]                         # [horizon, O, BC]
        out[c * BC : (c + 1) * BC] = yc.transpose(2, 0, 1)
    return out, res


def kernel(**inputs):
    out, _ = run(inputs, trace=False)
    return out
